# revision 4
# baseline (speedup 1.0000x reference)
"""TRN2 Bass kernel for nn_CombinedLoraA (moe_routing).

Computation: out[c, 0, r] = sum_k x[xids[c*64+r], 0, k] * A[wids[c], k, r]
  x: (512, 1, 4096) f32, xids: (20480,) i32, wids: (320,) i32, A: (80, 4096, 64) f32
  out: (320, 1, 64) f32

Strategy (adapter-parallel across 8 cores, routing baked in at trace time):
  - Host assigns exactly 10 adapters to each core (greedy row-count balance).
  - Each core computes the dense precompute P[w, t, r] = sum_k X[t, k] A[w, k, r]
    for ALL 512 tokens x its 10 adapters on the PE (X^T stationary, adapters'
    columns concatenated in the moving operand). X^T is transposed on the host.
  - P is copied PSUM->SBUF->DRAM in a [tc, p, w, r] layout; the needed
    out[c, r] = P[w_c, tok[c, r], r] elements are fetched with an indirect
    row-gather (64 f32 per row), then a one-hot mask multiply + free-axis
    reduce picks the diagonal element per (c, r) pair.
  - Host stitches the 8 per-core staging buffers into the (320, 1, 64) output.

Precision modes for the big matmul (error measured on HW at K=4096):
  float32 (3e-7), split bf16 hi/lo 3-matmul (4.5e-6), float32r (1.7e-4),
  bf16 (2.4e-3).
"""

import os
import sys

sys.path.insert(0, "/opt/trn_rl_repo")

import numpy as np
import ml_dtypes

import concourse.bass as bass
import concourse.tile as tile
from concourse import mybir, bacc
from concourse.bass import IndirectOffsetOnAxis
from concourse.bass_utils import run_bass_kernel_spmd

BATCH, C, R, K, NA = 512, 320, 64, 4096, 80
NCORES = 8
NW = NA // NCORES  # 10 adapters per core
KC = K // 128  # 32 contraction chunks
TC = BATCH // 128  # 4 token chunks
NFREE = NW * R  # 640 moving columns in the main matmul
PROWS = TC * 128 * NW  # 5120 rows in the P table

MODE = os.environ.get("KERNEL_MODE", "split")  # float32 | split | float32r | bf16


def _plan(wids: np.ndarray):
    """Assign exactly NW adapters per core, balancing total row count."""
    rows_of = [[] for _ in range(NA)]
    for c, w in enumerate(wids):
        rows_of[w].append(c)
    order = sorted(range(NA), key=lambda w: -len(rows_of[w]))
    core_adapters = [[] for _ in range(NCORES)]
    core_load = [0] * NCORES
    for w in order:
        cands = [i for i in range(NCORES) if len(core_adapters[i]) < NW]
        i = min(cands, key=lambda j: core_load[j])
        core_adapters[i].append(w)
        core_load[i] += len(rows_of[w])
    return rows_of, core_adapters


def _build_bass(nchunk: int):
    nc = bacc.Bacc("TRN2", target_bir_lowering=False)
    f32 = mybir.dt.float32
    bf16 = mybir.dt.bfloat16

    if MODE == "float32":
        din, halves, chunks = f32, 1, ((0, 512), (512, 640))
    elif MODE == "float32r":
        din, halves, chunks = mybir.dt.float32r, 1, ((0, 320), (320, 640))
    elif MODE == "bf16":
        din, halves, chunks = bf16, 1, ((0, 512), (512, 640))
    else:  # split
        din, halves, chunks = bf16, 2, ((0, 512), (512, 640))

    xt_d = [
        nc.dram_tensor(f"xt{h}", [K, BATCH], din, kind="ExternalInput")
        for h in range(halves)
    ]
    ac_d = [
        nc.dram_tensor(f"ac{h}", [KC, 128, NFREE], din, kind="ExternalInput")
        for h in range(halves)
    ]
    gidx_d = nc.dram_tensor("gidx", [128, nchunk], mybir.dt.int32, kind="ExternalInput")
    mask_d = nc.dram_tensor("mask", [128, R], f32, kind="ExternalInput")
    out_d = nc.dram_tensor("out", [128, nchunk], f32, kind="ExternalOutput")

    with tile.TileContext(nc) as tc:
        with (
            tc.tile_pool(name="big", bufs=1) as big,
            tc.tile_pool(name="work", bufs=2) as work,
            tc.tile_pool(name="ps", bufs=2, space="PSUM") as ps,
            tc.tile_pool(name="dram", bufs=1, space="DRAM") as dpool,
        ):
            pall = dpool.tile([PROWS, R], f32)

            # ---- resident loads: X^T and the concatenated adapter table ----
            xts, acs = [], []
            for h in range(halves):
                xt = big.tile([128, KC, BATCH], din, tag=f"xt{h}", name=f"xt{h}")
                # DMA in pieces of 4 kc (1-2 MiB each) for pipelining
                for i in range(0, KC, 4):
                    nc.sync.dma_start(
                        xt[:, i : i + 4, :],
                        xt_d[h]
                        .rearrange("(kc p) t -> p kc t", p=128)[:, i : i + 4, :],
                    )
                xts.append(xt)
                ac = big.tile([128, KC, NFREE], din, tag=f"ac{h}", name=f"ac{h}")
                for i in range(0, KC, 4):
                    nc.sync.dma_start(
                        ac[:, i : i + 4, :],
                        ac_d[h].rearrange("kc p n -> p kc n")[:, i : i + 4, :],
                    )
                acs.append(ac)

            gidx = big.tile([128, nchunk], mybir.dt.int32)
            nc.sync.dma_start(gidx[:], gidx_d[:])
            msk = big.tile([128, R], f32)
            nc.sync.dma_start(msk[:], mask_d[:])

            # ---- main matmul: P[tc] = X^T[:, tc].T @ A_cat, accum over kc ----
            pall_v = pall[:].rearrange("(tc p w) r -> tc p (w r)", tc=TC, p=128)
            for t in range(TC):
                psts = [
                    ps.tile([128, hi - lo], f32, tag=f"ps{ci}", name=f"ps{ci}")
                    for ci, (lo, hi) in enumerate(chunks)
                ]
                for kc in range(KC):
                    lhs = [xt[:, kc, t * 128 : (t + 1) * 128] for xt in xts]
                    rhs = [ac[:, kc, :] for ac in acs]
                    for ci, (lo, hi) in enumerate(chunks):
                        if halves == 1:
                            nc.tensor.matmul(
                                psts[ci][:],
                                lhs[0],
                                rhs[0][:, lo:hi],
                                start=(kc == 0),
                                stop=(kc == KC - 1),
                            )
                        else:  # split: hi*hi + hi*lo + lo*hi
                            for mi, (la, rb) in enumerate(((0, 0), (0, 1), (1, 0))):
                                nc.tensor.matmul(
                                    psts[ci][:],
                                    lhs[la],
                                    rhs[rb][:, lo:hi],
                                    start=(kc == 0 and mi == 0),
                                    stop=(kc == KC - 1 and mi == 2),
                                )
                pcopy = work.tile([128, NFREE], f32, tag="pcopy")
                for ci, (lo, hi) in enumerate(chunks):
                    nc.vector.tensor_copy(pcopy[:, lo:hi], psts[ci][:])
                nc.sync.dma_start(pall_v[t], pcopy[:])

            # ---- extraction: gather P rows, mask the diagonal, reduce ----
            g = big.tile([128, nchunk, R], f32)
            for j in range(nchunk):
                nc.gpsimd.indirect_dma_start(
                    out=g[:, j, :],
                    out_offset=None,
                    in_=pall[:],
                    in_offset=IndirectOffsetOnAxis(ap=gidx[:, j : j + 1], axis=0),
                )
            tmp = big.tile([128, nchunk, R], f32)
            nc.vector.tensor_tensor(
                out=tmp[:],
                in0=g[:],
                in1=msk[:].rearrange("p (o r) -> p o r", o=1).to_broadcast([128, nchunk, R]),
                op=mybir.AluOpType.mult,
            )
            outsb = big.tile([128, nchunk], f32)
            nc.vector.tensor_reduce(
                out=outsb[:],
                in_=tmp[:],
                axis=mybir.AxisListType.X,
                op=mybir.AluOpType.add,
            )
            nc.sync.dma_start(out_d[:], outsb[:])

    nc.compile()
    return nc


def _split_bf16(a: np.ndarray):
    hi = a.astype(ml_dtypes.bfloat16)
    lo = (a - hi.astype(np.float32)).astype(ml_dtypes.bfloat16)
    return hi, lo


def prepare(x, xids, wids, A):
    """Host-side planning + per-core input buffers. Returns (nc, in_maps, meta)."""
    x = np.ascontiguousarray(np.asarray(x).reshape(BATCH, K), dtype=np.float32)
    xids = np.asarray(xids).astype(np.int64)
    wids = np.asarray(wids).astype(np.int64)
    A = np.ascontiguousarray(np.asarray(A), dtype=np.float32)

    rows_of, core_adapters = _plan(wids)
    tok = xids.reshape(C, R)
    core_rows = [[c for w in ws for c in rows_of[w]] for ws in core_adapters]
    max_rows = max(len(r) for r in core_rows)
    nchunk = max(1, -(-max_rows * R // 128))  # ceil

    xt_f32 = np.ascontiguousarray(x.T)  # [K, BATCH]

    mask = np.zeros((128, R), dtype=np.float32)
    mask[np.arange(128), np.arange(128) % R] = 1.0

    in_maps = []
    for core in range(NCORES):
        ws = core_adapters[core]
        acore = A[ws]  # [NW, K, R]
        ac_f32 = np.ascontiguousarray(
            acore.transpose(1, 0, 2).reshape(KC, 128, NFREE)
        )

        slot = {w: i for i, w in enumerate(ws)}
        idx = np.zeros(nchunk * 128, dtype=np.int32)
        for i, c in enumerate(core_rows[core]):
            wl = slot[wids[c]]
            t = tok[c]  # (R,)
            idx[i * R : (i + 1) * R] = (t // 128) * (128 * NW) + (t % 128) * NW + wl
        gidx = np.ascontiguousarray(idx.reshape(nchunk, 128).T)  # [128, nchunk]

        m = {"gidx": gidx, "mask": mask}
        if MODE in ("float32", "float32r"):
            m["xt0"], m["ac0"] = xt_f32, ac_f32
        elif MODE == "bf16":
            m["xt0"] = xt_f32.astype(ml_dtypes.bfloat16)
            m["ac0"] = ac_f32.astype(ml_dtypes.bfloat16)
        else:  # split
            m["xt0"], m["xt1"] = _split_bf16(xt_f32)
            m["ac0"], m["ac1"] = _split_bf16(ac_f32)
        in_maps.append(m)

    nc = _build_bass(nchunk)
    return nc, in_maps, (core_rows, nchunk)


def finish(results, meta):
    core_rows, nchunk = meta
    out = np.zeros((C, 1, R), dtype=np.float32)
    for core in range(NCORES):
        vals = np.asarray(results[core]["out"]).T.reshape(-1)  # pair m = j*128+p
        rows = core_rows[core]
        out[rows, 0, :] = vals[: len(rows) * R].reshape(len(rows), R)
    return out


def kernel(x, xids, wids, A, _cache={}):
    nc, in_maps, meta = prepare(x, xids, wids, A)
    res = run_bass_kernel_spmd(nc, in_maps, core_ids=list(range(NCORES)))
    return finish(res.results, meta)


if __name__ == "__main__":
    # smoke test against a local numpy reference
    rng = np.random.default_rng(0)
    x = rng.standard_normal((BATCH, 1, K), dtype=np.float32)
    xids = rng.integers(0, BATCH, C * R).astype(np.int32)
    wids = rng.integers(0, NA, C).astype(np.int32)
    A = (rng.standard_normal((NA, K, R)) * 0.02).astype(np.float32)
    got = kernel(x=x, xids=xids, wids=wids, A=A)
    tokh = xids.reshape(C, R)
    want = np.einsum(
        "crk,ckr->cr",
        x[tokh, 0, :].astype(np.float64),
        A[wids].astype(np.float64),
    )[:, None, :]
    rel = np.abs(got - want).max() / np.abs(want).max()
    print(f"MODE={MODE} rel err vs f64: {rel:.3e}")
